# revision 1
# baseline (speedup 1.0000x reference)
"""DbrxExpertGLU (single-expert SwiGLU MLP) Trainium2 kernel.

  down = (silu(x @ w1.T) * (x @ v1.T)) @ w2
  x: [4096, 4096] f32, w1/v1/w2: [14336, 4096] f32 -> out [4096, 4096] f32

Strategy (8 NeuronCores, tensor-parallel over ffn dim per the expert-TP
hint): shard F=14336 into 8 x 1792. Each core computes gate/up/inter for
its F-shard and a partial down [4096, 4096]; the host sums the 8 fp32
partials (cheaper than an on-device all-reduce and off the HW critical
path).

On-device layout is activation-transposed ([feature, token]) so all three
matmuls chain with weights stationary and no transposes:
  gateT[f,t] = sum_h w1[f,h] x[t,h];  upT likewise
  interT     = sigmoid(gateT)*gateT*upT  (ACT+DVE, cast to bf16)
  downT[h,t] = sum_f w2[f,h] interT[f,t]
Matmuls run in bf16 (fp32 PSUM accumulation) -> PE at 1 cycle/row; the
whole kernel is PE-bound at ~98% of the bf16 roofline (~2.3 ms/core).
Host pre-casts/pre-tiles inputs so every DMA lands >=1KB-contiguous per
partition.
"""

import os
import subprocess
import sys
import tempfile
import time
from contextlib import ExitStack

import numpy as np
import ml_dtypes

import concourse.bass as bass
import concourse.mybir as mybir
import concourse.tile as tile
from concourse import bacc
from concourse.bass_utils import run_bass_kernel_spmd

BF16 = mybir.dt.bfloat16
F32 = mybir.dt.float32

T, H, F = 4096, 4096, 14336
N_CORES = 8
FS = F // N_CORES           # 1792 ffn rows per core
TC = 512                    # token chunk (= matmul moving dim)
NT, KB, FBN, HB = T // TC, H // 128, FS // 128, H // 128

_NC_CACHE = []


def _build():
    nc = bacc.Bacc("TRN2", target_bir_lowering=False, debug=False)

    xh = nc.dram_tensor("xh", [NT, KB, 128, TC], BF16, kind="ExternalInput").ap()
    w1h = nc.dram_tensor("w1h", [FBN, 128, KB, 128], BF16, kind="ExternalInput").ap()
    v1h = nc.dram_tensor("v1h", [FBN, 128, KB, 128], BF16, kind="ExternalInput").ap()
    w2h = nc.dram_tensor("w2h", [HB, 128, FBN, 128], BF16, kind="ExternalInput").ap()
    out = nc.dram_tensor("out", [H, T], F32, kind="ExternalOutput").ap()

    with tile.TileContext(nc) as tc, ExitStack() as ctx:
        xc_pool = ctx.enter_context(tc.tile_pool(name="xc", bufs=2))
        w1_pool = ctx.enter_context(tc.tile_pool(name="w1", bufs=3))
        v1_pool = ctx.enter_context(tc.tile_pool(name="v1", bufs=3))
        w2_pool = ctx.enter_context(tc.tile_pool(name="w2", bufs=3))
        inter_pool = ctx.enter_context(tc.tile_pool(name="inter", bufs=2))
        silu_pool = ctx.enter_context(tc.tile_pool(name="silu", bufs=3))
        out_pool = ctx.enter_context(tc.tile_pool(name="outp", bufs=4))
        pg_pool = ctx.enter_context(tc.tile_pool(name="pg", bufs=2, space="PSUM"))
        pu_pool = ctx.enter_context(tc.tile_pool(name="pu", bufs=2, space="PSUM"))
        pd_pool = ctx.enter_context(tc.tile_pool(name="pd", bufs=3, space="PSUM"))

        for tci in range(NT):
            # x chunk, free dim = (kb, t): rhs tiles for every h-block
            xc = xc_pool.tile([128, KB * TC], BF16)
            if tci == 0:
                # fine-grained first load on the otherwise-idle ACT HWDGE
                # ring (parallel to weight DMAs on SP) so the PE starts on
                # kb=0 ~13us sooner instead of waiting for the whole 4MB;
                # extra-fine leading slices, 4-kb steady slices
                bounds = [0, 2, 4] + list(range(8, KB + 1, 4))
                for k0, k1 in zip(bounds, bounds[1:]):
                    nc.scalar.dma_start(
                        out=xc[:, k0 * TC:k1 * TC].rearrange(
                            "p (kb t) -> p kb t", kb=k1 - k0
                        ),
                        in_=xh[tci, k0:k1].rearrange("kb p t -> p kb t"),
                    )
            else:
                nc.sync.dma_start(
                    out=xc[:].rearrange("p (kb t) -> p kb t", kb=KB),
                    in_=xh[tci].rearrange("kb p t -> p kb t"),
                )
            inter = inter_pool.tile([128, FBN * TC], BF16)

            # phase A: gateT/upT -> interT, one f-block (128 rows) at a time
            for fb in range(FBN):
                w1f = w1_pool.tile([128, KB * 128], BF16)
                if tci == 0 and fb == 0:
                    for k0 in range(0, KB, 8):
                        nc.sync.dma_start(
                            out=w1f[:, k0 * 128:(k0 + 8) * 128].rearrange(
                                "p (kb f) -> p kb f", kb=8
                            ),
                            in_=w1h[fb][:, k0:k0 + 8],
                        )
                else:
                    nc.sync.dma_start(
                        out=w1f[:].rearrange("p (kb f) -> p kb f", kb=KB), in_=w1h[fb]
                    )
                v1f = v1_pool.tile([128, KB * 128], BF16)
                nc.sync.dma_start(
                    out=v1f[:].rearrange("p (kb f) -> p kb f", kb=KB), in_=v1h[fb]
                )
                pg = pg_pool.tile([128, TC], F32)
                pu = pu_pool.tile([128, TC], F32)
                for kb in range(KB):
                    nc.tensor.matmul(
                        pg[:], w1f[:, bass.ts(kb, 128)], xc[:, bass.ts(kb, TC)],
                        start=(kb == 0), stop=(kb == KB - 1),
                    )
                for kb in range(KB):
                    nc.tensor.matmul(
                        pu[:], v1f[:, bass.ts(kb, 128)], xc[:, bass.ts(kb, TC)],
                        start=(kb == 0), stop=(kb == KB - 1),
                    )
                sg = silu_pool.tile([128, TC], F32)
                nc.scalar.activation(
                    sg[:], pg[:], mybir.ActivationFunctionType.Sigmoid
                )
                sl = silu_pool.tile([128, TC], F32)
                nc.vector.tensor_mul(sl[:], sg[:], pg[:])
                nc.vector.tensor_mul(inter[:, bass.ts(fb, TC)], sl[:], pu[:])

            # phase B: partial downT, one h-block at a time
            for hb in range(HB):
                w2t = w2_pool.tile([128, FBN * 128], BF16)
                nc.sync.dma_start(
                    out=w2t[:].rearrange("p (fb h) -> p fb h", fb=FBN), in_=w2h[hb]
                )
                # final output tile: two N=256 groups (same PE cycles) so the
                # first half's copy+DMA-out hides under the second half's
                # matmuls instead of dangling off the kernel tail
                last_tile = tci == NT - 1 and hb == HB - 1
                splits = (0, 256, 384, 512) if last_tile else (0, TC)
                for si in range(len(splits) - 1):
                    c0, c1 = splits[si], splits[si + 1]
                    pd = pd_pool.tile([128, c1 - c0], F32)
                    for fb in range(FBN):
                        nc.tensor.matmul(
                            pd[:], w2t[:, bass.ts(fb, 128)],
                            inter[:, fb * TC + c0:fb * TC + c1],
                            start=(fb == 0), stop=(fb == FBN - 1),
                        )
                    ob = out_pool.tile([128, c1 - c0], F32)
                    nc.scalar.copy(ob[:], pd[:])
                    nc.sync.dma_start(
                        out=out[hb * 128:(hb + 1) * 128,
                                tci * TC + c0:tci * TC + c1],
                        in_=ob[:],
                    )

    nc.compile()
    return nc


def _prep_inputs(x, w1, v1, w2):
    bf = ml_dtypes.bfloat16
    # x[t, h] -> xh[tc, kb, p(h%128), tt]
    xh = np.ascontiguousarray(
        x.astype(bf).reshape(NT, TC, KB, 128).transpose(0, 2, 3, 1)
    )
    in_maps = []
    for c in range(N_CORES):
        sl = slice(c * FS, (c + 1) * FS)
        w1s = w1[sl].astype(bf)
        v1s = v1[sl].astype(bf)
        w2s = w2[sl].astype(bf)
        in_maps.append({
            "xh": xh,
            # w1[f, h] -> [fb, p(h%128), kb, ff]
            "w1h": np.ascontiguousarray(
                w1s.reshape(FBN, 128, KB, 128).transpose(0, 3, 2, 1)
            ),
            "v1h": np.ascontiguousarray(
                v1s.reshape(FBN, 128, KB, 128).transpose(0, 3, 2, 1)
            ),
            # w2[f, h] -> [hb, p(f%128), fb, hh]
            "w2h": np.ascontiguousarray(
                w2s.reshape(FBN, 128, HB, 128).transpose(2, 1, 0, 3)
            ),
        })
    return in_maps


def _exec_once(in_maps):
    """One 8-core device execution; returns summed partial [H, T] f32."""
    if not _NC_CACHE:
        _NC_CACHE.append(_build())
    res = run_bass_kernel_spmd(_NC_CACHE[0], in_maps, list(range(N_CORES)))
    acc = res.results[0]["out"].astype(np.float32)
    for c in range(1, N_CORES):
        acc += res.results[c]["out"]
    if not np.isfinite(acc).all():
        raise FloatingPointError("non-finite output from device")
    return acc


def _exec_subprocess(in_maps):
    """Retry path: run the device execution in a fresh process (fresh axon
    client) in case this process's device session is poisoned."""
    base = "/dev/shm" if os.path.isdir("/dev/shm") else None
    with tempfile.TemporaryDirectory(dir=base) as d:
        np.save(os.path.join(d, "xh.npy"), in_maps[0]["xh"].view(np.uint16))
        for c, m in enumerate(in_maps):
            for k in ("w1h", "v1h", "w2h"):
                np.save(os.path.join(d, f"{k}_{c}.npy"), m[k].view(np.uint16))
        subprocess.run(
            [sys.executable, os.path.abspath(__file__), "--subproc", d],
            check=True, timeout=1200,
        )
        return np.load(os.path.join(d, "acc.npy"))


def _subproc_main(d):
    bf = ml_dtypes.bfloat16
    xh = np.load(os.path.join(d, "xh.npy")).view(bf)
    in_maps = []
    for c in range(N_CORES):
        m = {"xh": xh}
        for k in ("w1h", "v1h", "w2h"):
            m[k] = np.load(os.path.join(d, f"{k}_{c}.npy")).view(bf)
        in_maps.append(m)
    np.save(os.path.join(d, "acc.npy"), _exec_once(in_maps))


def kernel(x, expert_w1, expert_v1, expert_w2):
    x = np.asarray(x, dtype=np.float32)
    expert_w1 = np.asarray(expert_w1, dtype=np.float32)
    expert_v1 = np.asarray(expert_v1, dtype=np.float32)
    expert_w2 = np.asarray(expert_w2, dtype=np.float32)
    assert x.shape == (T, H) and expert_w1.shape == (F, H)

    in_maps = _prep_inputs(x, expert_w1, expert_v1, expert_w2)

    acc = None
    last_err = None
    for attempt in range(4):
        try:
            if attempt < 2:
                acc = _exec_once(in_maps)
            else:
                acc = _exec_subprocess(in_maps)
            break
        except Exception as e:  # transient device/tunnel errors: retry
            last_err = e
            time.sleep(3.0)
    if acc is None:
        raise last_err
    return np.ascontiguousarray(acc.T)  # [h, t] -> [t, h]


if __name__ == "__main__" and len(sys.argv) == 3 and sys.argv[1] == "--subproc":
    _subproc_main(sys.argv[2])



# revision 3
# speedup vs baseline: 1.2945x; 1.2945x over previous
"""DbrxExpertGLU (single-expert SwiGLU MLP) Trainium2 kernel.

  down = (silu(x @ w1.T) * (x @ v1.T)) @ w2
  x: [4096, 4096] f32, w1/v1/w2: [14336, 4096] f32 -> out [4096, 4096] f32

Strategy (8 NeuronCores, tensor-parallel over ffn dim per the expert-TP
hint): shard F=14336 into 8 x 1792. Each core computes gate/up/inter for
its F-shard and a partial down [4096, 4096]; the host sums the 8 fp16
partials.

All three matmuls run in fp8(e4m3) DoubleRow mode (0.5 PE cycles per
output column, K=256 per call -> 4x the bf16 MAC rate) with a 3-term
error-compensated split per operand pair:

    A @ B ~= Ah@Bh + Al@Bh + Ah@Bl        (A = Ah + Al, fp8 hi/lo split)

The two cross terms ride in ONE DoubleRow call per 128-K block (slab0 =
(Bh, Al), slab1 = (Bl, Ah)), the hi*hi term paces K=256 per call, so a
logical matmul costs 0.75x its bf16 time while keeping ~0.2% rel err
(validated vs numpy: pure fp8 is 6.6%, any 2-term variant >2.6%). All
three terms accumulate in one PSUM group at natural scale (fp8 is
floating point; lo magnitudes ~6% of hi need no rescale).

Layout per core: activation-transposed chains ([feature, token]); hi/lo
planes interleaved per 128-K block (k-major [kb, hl, cols]) so every
matmul AP stride stays <= 2048 elements (walrus's step_elem field is
signed 16-bit; plane-major layouts overflow it at KB*SC = 32768).
Tokens go in 4 super-chunks of 1024 (weights stream once per
super-chunk -> ~240MB total DMA under the ~1.72ms PE roofline).
"""

import os
import subprocess
import sys
import tempfile
import time
from contextlib import ExitStack

import numpy as np
import ml_dtypes

import concourse.bass as bass
import concourse.mybir as mybir
import concourse.tile as tile
from concourse import bacc
from concourse.bass_utils import run_bass_kernel_spmd

F8 = mybir.dt.float8e4
F16 = mybir.dt.float16
F32 = mybir.dt.float32
NPF8 = ml_dtypes.float8_e4m3
DR = mybir.MatmulPerfMode.DoubleRow
ACT = mybir.ActivationFunctionType

T, H, F = 4096, 4096, 14336
N_CORES = 8
FS = F // N_CORES           # 1792 ffn rows per core
FBN = FS // 128             # 14 f-blocks
KB = H // 128               # 32 k-blocks (hidden contraction)
HB = H // 128               # 32 h-blocks (down-proj output rows)
TC = 512                    # matmul moving width / PSUM tile
SC = 1024                   # token super-chunk (weights stream once per SC)
NSC = T // SC               # 4 super-chunks
NIC = SC // TC              # 2 inner chunks

_NC_CACHE = {}


def _build(sg_inv, c_pu, out_scale):
    nc = bacc.Bacc("TRN2", target_bir_lowering=False, debug=False)

    # hi/lo interleaved per k-block: x/inter planes (lo, hi); weights
    # (hi, lo) -> the cross-term DoubleRow call pairs slab0=(Wh, Xl),
    # slab1=(Wl, Xh) with stride one plane.
    xd = nc.dram_tensor("xd", [NSC, 128, KB, 2, SC], F8, kind="ExternalInput").ap()
    w1d = nc.dram_tensor("w1d", [FBN, 128, KB, 2, 128], F8, kind="ExternalInput").ap()
    v1d = nc.dram_tensor("v1d", [FBN, 128, KB, 2, 128], F8, kind="ExternalInput").ap()
    w2d = nc.dram_tensor("w2d", [HB, 128, FBN, 2, 128], F8, kind="ExternalInput").ap()
    out = nc.dram_tensor("out", [H, T], F16, kind="ExternalOutput").ap()

    with tile.TileContext(nc) as tc, ExitStack() as ctx:
        xc_pool = ctx.enter_context(tc.tile_pool(name="xc", bufs=1))
        w1_pool = ctx.enter_context(tc.tile_pool(name="w1", bufs=3))
        v1_pool = ctx.enter_context(tc.tile_pool(name="v1", bufs=3))
        w2_pool = ctx.enter_context(tc.tile_pool(name="w2", bufs=3))
        inter_pool = ctx.enter_context(tc.tile_pool(name="inter", bufs=1))
        eps_pool = ctx.enter_context(tc.tile_pool(name="eps", bufs=3))
        out_pool = ctx.enter_context(tc.tile_pool(name="outp", bufs=4))
        pg_pool = ctx.enter_context(tc.tile_pool(name="pg", bufs=2, space="PSUM"))
        pu_pool = ctx.enter_context(tc.tile_pool(name="pu", bufs=2, space="PSUM"))
        pd_pool = ctx.enter_context(tc.tile_pool(name="pd", bufs=3, space="PSUM"))

        for sc in range(NSC):
            xc = xc_pool.tile([128, KB, 2, SC], F8)
            if sc == 0:
                # fine-grained first load on the otherwise-idle ACT ring
                # (parallel to weight DMAs on SP) so the PE starts sooner.
                bounds = [0, 2, 4] + list(range(8, KB + 1, 4))
                for k0, k1 in zip(bounds, bounds[1:]):
                    nc.scalar.dma_start(out=xc[:, k0:k1], in_=xd[sc, :, k0:k1])
            else:
                nc.sync.dma_start(out=xc[:, 0:KB // 2], in_=xd[sc, :, 0:KB // 2])
                nc.sync.dma_start(out=xc[:, KB // 2:], in_=xd[sc, :, KB // 2:])

            inter = inter_pool.tile([128, FBN, 2, SC], F8)

            # ---- phase A: gateT/upT -> interT, one f-block at a time ----
            for fb in range(FBN):
                w1f = w1_pool.tile([128, KB, 2, 128], F8)
                if sc == 0 and fb == 0:
                    for k0 in range(0, KB, 8):
                        nc.sync.dma_start(out=w1f[:, k0:k0 + 8],
                                          in_=w1d[fb, :, k0:k0 + 8])
                else:
                    nc.sync.dma_start(out=w1f[:], in_=w1d[fb])
                v1f = v1_pool.tile([128, KB, 2, 128], F8)
                nc.sync.dma_start(out=v1f[:], in_=v1d[fb])

                for ic in range(NIC):
                    c0, c1 = ic * TC, (ic + 1) * TC

                    def mm3(psum, wf):
                        # hi*hi: kb pairs, slabs = (w_hi[kb], x_hi[kb])
                        for kbp in range(0, KB, 2):
                            nc.tensor.matmul(
                                psum[:], wf[:, kbp:kbp + 2, 0],
                                xc[:, kbp:kbp + 2, 1, c0:c1],
                                start=(kbp == 0), stop=False, perf_mode=DR)
                        # cross: slab0 = (w_hi, x_lo), slab1 = (w_lo, x_hi)
                        for kb in range(KB):
                            nc.tensor.matmul(
                                psum[:], wf[:, kb], xc[:, kb, :, c0:c1],
                                start=False, stop=(kb == KB - 1), perf_mode=DR)

                    pg = pg_pool.tile([128, TC], F32)
                    mm3(pg, w1f)
                    pu = pu_pool.tile([128, TC], F32)
                    mm3(pu, v1f)

                    sl = eps_pool.tile([128, TC], F32)
                    nc.scalar.activation(sl[:], pg[:], ACT.Silu, scale=sg_inv)
                    pus = eps_pool.tile([128, TC], F32)
                    nc.scalar.mul(pus[:], pu[:], c_pu)
                    t = eps_pool.tile([128, TC], F32)
                    nc.vector.tensor_mul(t[:], sl[:], pus[:])
                    nc.scalar.copy(inter[:, fb, 1, c0:c1], t[:])
                    nc.vector.tensor_sub(inter[:, fb, 0, c0:c1], t[:],
                                         inter[:, fb, 1, c0:c1])

            # ---- phase B: partial downT, one h-block at a time ----
            for hb in range(HB):
                w2t = w2_pool.tile([128, FBN, 2, 128], F8)
                nc.sync.dma_start(out=w2t[:], in_=w2d[hb])
                for ic in range(NIC):
                    c0, c1 = ic * TC, (ic + 1) * TC
                    pd = pd_pool.tile([128, TC], F32)
                    for fbp in range(0, FBN, 2):
                        nc.tensor.matmul(
                            pd[:], w2t[:, fbp:fbp + 2, 0],
                            inter[:, fbp:fbp + 2, 1, c0:c1],
                            start=(fbp == 0), stop=False, perf_mode=DR)
                    for fb in range(FBN):
                        nc.tensor.matmul(
                            pd[:], w2t[:, fb], inter[:, fb, :, c0:c1],
                            start=False, stop=(fb == FBN - 1), perf_mode=DR)
                    ob = out_pool.tile([128, TC], F16)
                    nc.scalar.mul(ob[:], pd[:], out_scale)
                    nc.sync.dma_start(
                        out=out[hb * 128:(hb + 1) * 128,
                                sc * SC + c0:sc * SC + c1],
                        in_=ob[:])

    nc.compile()
    return nc


def _pow2_scale(a, target=224.0):
    m = float(np.abs(a).max())
    if m == 0.0 or not np.isfinite(m):
        return 1.0
    return float(2.0 ** np.floor(np.log2(target / m)))


def _split(a):
    """fp8 e4m3 hi/lo decomposition of an f32 array (already scaled)."""
    hi = a.astype(NPF8)
    lo = (a - hi.astype(np.float32)).astype(NPF8)
    return hi, lo


def _prep(x, w1, v1, w2):
    sx = _pow2_scale(x)
    sw1 = _pow2_scale(w1)
    sv1 = _pow2_scale(v1)
    sw2 = _pow2_scale(w2)

    # inter scale: estimate absmax(silu(gate)*up) from a 128-token sample,
    # then leave ~8x headroom below fp8 max (240).
    xs_sample = x[:: T // 128][:128]
    gs = xs_sample @ w1.T
    us = xs_sample @ v1.T
    inter_s = (gs / (1.0 + np.exp(-np.clip(gs, -30, 30)))) * us
    est = float(np.abs(inter_s).max())
    si = float(2.0 ** np.floor(np.log2(28.0 / max(est, 1e-6))))

    sg_inv = 1.0 / (sx * sw1)
    c_pu = si / (sx * sv1)
    out_scale = 1.0 / (si * sw2)

    # x[t, h] scaled -> [sc, p(h%128), kb, 2(lo,hi), tt]
    xh, xl = _split(x * sx)

    def pack_x(a):
        return a.reshape(NSC, SC, KB, 128).transpose(0, 3, 2, 1)

    xd = np.empty((NSC, 128, KB, 2, SC), dtype=NPF8)
    xd[:, :, :, 0] = pack_x(xl)
    xd[:, :, :, 1] = pack_x(xh)

    in_maps = []
    for c in range(N_CORES):
        rows = slice(c * FS, (c + 1) * FS)
        w1h, w1l = _split(w1[rows] * sw1)
        v1h, v1l = _split(v1[rows] * sv1)
        w2h, w2l = _split(w2[rows] * sw2)

        def pack_w(a):
            # [FS, H] -> [fb, p(h%128), kb, f']
            return a.reshape(FBN, 128, KB, 128).transpose(0, 3, 2, 1)

        w1p = np.empty((FBN, 128, KB, 2, 128), dtype=NPF8)
        w1p[:, :, :, 0] = pack_w(w1h)
        w1p[:, :, :, 1] = pack_w(w1l)
        v1p = np.empty((FBN, 128, KB, 2, 128), dtype=NPF8)
        v1p[:, :, :, 0] = pack_w(v1h)
        v1p[:, :, :, 1] = pack_w(v1l)

        def pack_w2(a):
            # [FS, H] -> [hb, p(f%128), fb, h']
            return a.reshape(FBN, 128, HB, 128).transpose(2, 1, 0, 3)

        w2p = np.empty((HB, 128, FBN, 2, 128), dtype=NPF8)
        w2p[:, :, :, 0] = pack_w2(w2h)
        w2p[:, :, :, 1] = pack_w2(w2l)

        in_maps.append({"xd": xd, "w1d": w1p, "v1d": v1p, "w2d": w2p})

    return in_maps, (sg_inv, c_pu, out_scale)


def _exec_once(in_maps, scales):
    """One 8-core device execution; returns summed partial [H, T] f32."""
    if scales not in _NC_CACHE:
        _NC_CACHE[scales] = _build(*scales)
    res = run_bass_kernel_spmd(_NC_CACHE[scales], in_maps, list(range(N_CORES)))
    acc = res.results[0]["out"].astype(np.float32)
    for c in range(1, N_CORES):
        acc += res.results[c]["out"].astype(np.float32)
    if not np.isfinite(acc).all():
        raise FloatingPointError("non-finite output from device")
    return acc


def _exec_subprocess(in_maps, scales):
    """Retry path: run the device execution in a fresh process (fresh axon
    client) in case this process's device session is poisoned."""
    base = "/dev/shm" if os.path.isdir("/dev/shm") else None
    with tempfile.TemporaryDirectory(dir=base) as d:
        np.save(os.path.join(d, "scales.npy"), np.array(scales, dtype=np.float64))
        np.save(os.path.join(d, "xd.npy"), in_maps[0]["xd"].view(np.uint8))
        for c, m in enumerate(in_maps):
            for k in ("w1d", "v1d", "w2d"):
                np.save(os.path.join(d, f"{k}_{c}.npy"), m[k].view(np.uint8))
        subprocess.run(
            [sys.executable, os.path.abspath(__file__), "--subproc", d],
            check=True, timeout=1800,
        )
        return np.load(os.path.join(d, "acc.npy"))


def _subproc_main(d):
    scales = tuple(np.load(os.path.join(d, "scales.npy")).tolist())
    xd = np.load(os.path.join(d, "xd.npy")).view(NPF8)
    in_maps = []
    for c in range(N_CORES):
        m = {"xd": xd}
        for k in ("w1d", "v1d", "w2d"):
            m[k] = np.load(os.path.join(d, f"{k}_{c}.npy")).view(NPF8)
        in_maps.append(m)
    np.save(os.path.join(d, "acc.npy"), _exec_once(in_maps, scales))


def kernel(x, expert_w1, expert_v1, expert_w2):
    x = np.asarray(x, dtype=np.float32)
    expert_w1 = np.asarray(expert_w1, dtype=np.float32)
    expert_v1 = np.asarray(expert_v1, dtype=np.float32)
    expert_w2 = np.asarray(expert_w2, dtype=np.float32)
    assert x.shape == (T, H) and expert_w1.shape == (F, H)

    in_maps, scales = _prep(x, expert_w1, expert_v1, expert_w2)

    acc = None
    last_err = None
    for attempt in range(4):
        try:
            if attempt < 2:
                acc = _exec_once(in_maps, scales)
            else:
                acc = _exec_subprocess(in_maps, scales)
            break
        except Exception as e:  # transient device/tunnel errors: retry
            last_err = e
            time.sleep(3.0)
    if acc is None:
        raise last_err
    return np.ascontiguousarray(acc.T)  # [h, t] -> [t, h]


if __name__ == "__main__" and len(sys.argv) == 3 and sys.argv[1] == "--subproc":
    _subproc_main(sys.argv[2])
